# revision 69
# baseline (speedup 1.0000x reference)
"""Causal multi-head attention (QKV-packed) on 8 Trainium2 NeuronCores.

Sharding: pure head-parallel. B*H = 32 (batch, head) pairs -> 4 per core,
zero inter-core communication. Per head, flash-style causal attention in
the transposed orientation (no on-device transposes):

  - Host pre-lays-out Q^T, K^T as bf16 [D=128, S] (D on partitions) and V
    as k-blocks [128, D]; scores S_T[k, q] = (K^T_j).T @ Q^T land in
    multi-bank PSUM "group" tiles so ONE activation instruction
    exponentiates several k-blocks at once (the ACT engine pays a fixed
    ~185ns access penalty per instruction).
  - Causal masking is multiplicative post-exp (0/1 bf16 masks) on the
    otherwise-idle Pool/GPSIMD engine.
  - The softmax denominator: full-width k-blocks are accumulated into a
    per-strip bf16 acc[128,512] on the DVE (2x bf16 mode), which a single
    all-ones [128x128] matmul reduces AND broadcasts into PSUM; the
    diagonal partial blocks skip the DVE and contribute via direct
    all-ones matmuls that PSUM-accumulate on top. This keeps the PE free
    of the O(S^2) ones-matmul streaming the baseline paid per block.
  - Normalization: DVE reciprocal of the broadcast denominator, one DVE
    multiply, DMA out. Output returned as O^T, un-transposed on host.
"""

import sys

if "/opt/trn_rl_repo" not in sys.path:
    sys.path.insert(0, "/opt/trn_rl_repo")

import numpy as np

B, S, H, D = 2, 2048, 16, 128
NCORES = 8
HPC = (B * H) // NCORES  # heads per core = 4
QS = 512   # q-strip width (PSUM bank)
KB = 128   # k-block (partition dim)
SCALE = 1.0 / float(np.sqrt(D))
NSTRIP = S // QS  # 4
BANK = 512  # PSUM bank width in f32 elements
CAPS = (1536, 1024)  # alternating score-group tile widths (3 banks, 2 banks)

_nc_cache = {}


def strip_plan(s, start_idx=0):
    """Pack strip s's k-blocks into alternating multi-bank PSUM group
    tiles, starting with tag `start_idx`. Returns list of groups:
    (cap_idx, used_cols, blocks) with blocks = [(col, j, off, w, diag)];
    col = column inside the group tile, off = q-offset inside the strip,
    w = matmul/exp width.

    Full blocks (t = j-4s <= 0) are 512 wide at off 0; t=0 is full width
    but causal-masked in its first 128 columns. Partials: t=1 (off 128,
    w 384), t=2 (off 256, w 256), t=3 (off 384, w 128), all masked in
    their first 128 columns. Pack order puts the 128-wide t=3 right
    after t=1's 384 so they share a bank with no gap.
    """
    blocks = [(j, 0, 512, j == 4 * s) for j in range(4 * s + 1)]
    blocks += [
        (4 * s + 1, 128, 384, True),
        (4 * s + 3, 384, 128, True),
        (4 * s + 2, 256, 256, True),
    ]
    groups = []
    cap_idx, col, cur = start_idx, 0, []
    for j, off, w, diag in blocks:
        cap = CAPS[cap_idx]
        c = col
        if c // BANK != (c + w - 1) // BANK:
            c = (c // BANK + 1) * BANK  # don't cross a bank boundary
        if c + w > cap:
            groups.append((cap_idx, col, cur))
            cap_idx, c, cur = (cap_idx + 1) % 2, 0, []
        cur.append((c, j, off, w, diag))
        col = c + w
    groups.append((cap_idx, col, cur))
    return groups


def _build_nc():
    import concourse.bass as bass  # noqa: F401
    import concourse.mybir as mybir
    from concourse import bacc
    from concourse.tile import TileContext

    f32 = mybir.dt.float32
    bf16 = mybir.dt.bfloat16
    Exp = mybir.ActivationFunctionType.Exp

    nc = bacc.Bacc()
    # One packed input per head, bf16 [128, 3*S]: cols [0,S) = Q^T,
    # [S,2S) = K^T, [2S,3S) = V swizzled (v[p, j*KB+d] = V[j*KB+p, d]).
    qkvT = nc.declare_dram_parameter("qkvT", [HPC, 128, 3 * S], bf16, isOutput=False)
    # cst bf16 [128, 256]: [:, 0:128] = tri01 (1 if dk<=c else 0),
    # [:, 128:256] = all-ones.
    cst = nc.declare_dram_parameter("cst", [128, 256], bf16, isOutput=False)
    oT = nc.declare_dram_parameter("oT", [HPC, 128, S], bf16, isOutput=True)

    # Strip processing order: small strip first on head 0 (fast pipeline
    # fill); the final head ends on strip 1 so only one strip epilogue
    # remains after the last exp.
    strip_orders = [[0, 1, 2, 3], [1, 3, 2, 0], [1, 3, 2, 0], [3, 2, 1, 0]]

    with TileContext(nc) as tc:
        with (
            nc.allow_low_precision(
                reason="bf16 P/V/acc; softmax weights tolerate 2^-9"
            ),
            tc.tile_pool(name="cpool", bufs=1) as cpool,
            tc.tile_pool(name="qkpool", bufs=2) as qkpool,
            tc.tile_pool(name="ptpool", bufs=4) as ptpool,
            tc.tile_pool(name="acpool", bufs=2) as acpool,
            tc.tile_pool(name="obpool", bufs=2) as obpool,
            tc.tile_pool(name="psg", bufs=1, space="PSUM") as psg,
            tc.tile_pool(name="pso", bufs=2, space="PSUM") as pso,
            tc.tile_pool(name="psd", bufs=1, space="PSUM") as psd,
        ):
            cst_sb = cpool.tile([128, 256], bf16)
            tri01 = cst_sb[:, 0:128]
            ones_bf = cst_sb[:, 128:256]

            def consume(st):
                """DVE mask+accumulate + PE O-matmuls for a finished group.
                Diagonal blocks are masked first (DVE 0/1 multiply) and
                their O-matmuls emitted last so the non-diagonal matmuls
                never wait on the mask."""
                (s, blocks, pt, vv, o_ps, acc, first_full, ocount, nmm) = st
                for col, j, off, w, diag in blocks:
                    if diag:
                        nc.vector.tensor_mul(
                            pt[:, col : col + 128], pt[:, col : col + 128], tri01
                        )
                for col, j, off, w, diag in blocks:
                    if w == 512:
                        if first_full[0]:
                            nc.vector.tensor_copy(acc[:], pt[:, col : col + 512])
                            first_full[0] = False
                        else:
                            nc.vector.tensor_add(
                                acc[:], acc[:], pt[:, col : col + 512]
                            )
                for col, j, off, w, diag in sorted(blocks, key=lambda b: b[4]):
                    nc.tensor.matmul(
                        o_ps[:, off : off + w],
                        lhsT=vv[:, KB * j : KB * (j + 1)],
                        rhs=pt[:, col : col + w],
                        start=ocount[0] == 0,
                        stop=ocount[0] == nmm - 1,
                    )
                    ocount[0] += 1

            def fin_den(acc, pdens, last=False):
                """Denominator reduce+broadcast (PE): all-ones matmul on the
                DVE accumulator, PSUM-accumulating the diagonal partials on
                top. Emitted BEFORE the group's O-matmuls so the reciprocal
                overlaps them."""
                if last:
                    # the score-group banks are idle at the drain tail; borrow
                    # one so this epilogue doesn't serialize on the den bank
                    # behind the previous strip's reciprocal
                    den_wide = psg.tile([128, CAPS[1]], f32, tag="sg1")
                    den_ps = den_wide[:, 0:QS]
                else:
                    den_ps = psd.tile([128, QS], f32, tag="den")
                nc.tensor.matmul(
                    den_ps[:], lhsT=ones_bf, rhs=acc[:], start=True, stop=False
                )
                for i, (pt, col, off, w) in enumerate(pdens):
                    nc.tensor.matmul(
                        den_ps[:, off : off + w],
                        lhsT=ones_bf,
                        rhs=pt[:, col : col + w],
                        start=False,
                        stop=(i == len(pdens) - 1),
                    )
                return den_ps

            def fin_norm(h, s, o_ps, den_ps, last=False):
                """Normalize (single DVE divide) and store. The final strip's
                store is dispatched from the (idle-by-then) ACT sequencer so
                it does not queue behind the previous store's wait on the SP
                sequencer."""
                recip = obpool.tile([128, QS], f32, tag="recip")
                nc.vector.reciprocal(recip[:], den_ps[:])
                o_sb = obpool.tile([128, QS], bf16, tag="o_sb")
                nc.vector.tensor_mul(o_sb[:], o_ps[:], recip[:])
                eng = nc.sync
                eng.dma_start(out=oT[h][:, QS * s : QS * (s + 1)], in_=o_sb[:])

            pend_q = []  # consume queue, depth 2: O-matmuls trail 2 groups
            qkv_tiles = {}

            def load_head(hh, first=False):
                t = qkpool.tile([128, 3 * S], bf16, tag="qkv_sb")
                qkv_tiles[hh] = t
                if first:
                    # split the first head's load so the first exp fires as
                    # early as possible: Q^T strip 0 on the ACT sequencer
                    # (parallel HWDGE dispatch) while SP sends K^T block 0
                    # (128 cols — all the first 1-block group needs), then
                    # the rest in need order
                    nc.scalar.dma_start(out=t[:, 0:512], in_=qkvT[hh][:, 0:512])
                    for c0, c1 in (
                        (S, S + 512),
                        (2 * S, 2 * S + 512),
                    ):
                        nc.sync.dma_start(out=t[:, c0:c1], in_=qkvT[hh][:, c0:c1])
                    nc.sync.dma_start(out=cst_sb[:], in_=cst[:])
                    for c0, c1 in (
                        (512, S),
                        (S + 512, 2 * S),
                        (2 * S + 512, 3 * S),
                    ):
                        nc.sync.dma_start(out=t[:, c0:c1], in_=qkvT[hh][:, c0:c1])
                else:
                    # 3 chunks so output stores interleave at the DMA engines
                    for c0, c1 in ((S, 2 * S), (0, S), (2 * S, 3 * S)):
                        nc.sync.dma_start(out=t[:, c0:c1], in_=qkvT[hh][:, c0:c1])

            # Flat block stream over (head, strip): groups pack ACROSS strip
            # and head boundaries so the exp pipeline never drains at a
            # boundary. Each entry: (h, s, j, off, w, diag, prefetch,
            # strip_done). Fulls first, then partials packed 384/128/256.
            stream = []
            for h in range(HPC):
                for si, s in enumerate(strip_orders[h]):
                    blk = [(j, 0, 512, j == 4 * s) for j in range(4 * s + 1)]
                    blk += [
                        (4 * s + 1, 128, 384, True),
                        (4 * s + 3, 384, 128, True),
                        (4 * s + 2, 256, 256, True),
                    ]
                    for bi, (j, off, w, diag) in enumerate(blk):
                        stream.append(
                            (h, s, j, off, w, diag,
                             bi == 0 and si == 1 and h + 1 < HPC,
                             bi == len(blk) - 1)
                        )
            # greedy pack into alternating 3-bank / 2-bank PSUM group tiles.
            # A block that would have to skip ahead to the next bank boundary
            # closes the group instead: gaps would force the exp to split
            # into spans, costing an extra ACT instruction each.
            groups = []
            cap_idx, col, cur = 0, 0, []
            for ent in stream:
                w = ent[4]
                cap = CAPS[cap_idx]
                c = col
                if c // BANK != (c + w - 1) // BANK:
                    c = (c // BANK + 1) * BANK
                if c + w > cap:
                    groups.append((cap_idx, col, cur))
                    cap_idx, c, cur = (cap_idx + 1) % 2, 0, []
                cur.append((c,) + ent)
                col = c + w
            groups.append((cap_idx, col, cur))

            strip_states = {}

            def get_state(h, s):
                if (h, s) not in strip_states:
                    o_ps = pso.tile([128, QS], f32, tag="o_ps")
                    acc = acpool.tile([128, QS], bf16, tag="acc")
                    strip_states[(h, s)] = {
                        "o_ps": o_ps, "acc": acc, "first_full": True,
                        "ocount": 0, "nmm": 4 * s + 4, "pdens": [],
                    }
                return strip_states[(h, s)]

            load_head(0, first=True)
            last_hs = (HPC - 1, strip_orders[-1][-1])

            def consume(st, tail=False):
                blocks, pt = st
                for col, h, s, j, off, w, diag, pf, sd in blocks:
                    if diag:
                        nc.vector.tensor_mul(
                            pt[:, col : col + 128], pt[:, col : col + 128], tri01
                        )
                for col, h, s, j, off, w, diag, pf, sd in blocks:
                    if w == 512:
                        stt = strip_states[(h, s)]
                        if stt["first_full"]:
                            nc.vector.tensor_copy(
                                stt["acc"][:], pt[:, col : col + 512]
                            )
                            stt["first_full"] = False
                        else:
                            nc.vector.tensor_add(
                                stt["acc"][:], stt["acc"][:],
                                pt[:, col : col + 512],
                            )
                fins = [(h, s) for col, h, s, j, off, w, diag, pf, sd in blocks
                        if sd]
                dens = []
                if tail:
                    for h, s in fins:
                        stt = strip_states[(h, s)]
                        dens.append(
                            fin_den(stt["acc"], stt["pdens"],
                                    last=(h, s) == last_hs)
                        )
                for col, h, s, j, off, w, diag, pf, sd in sorted(
                    blocks, key=lambda b: b[6]
                ):
                    stt = strip_states[(h, s)]
                    nc.tensor.matmul(
                        stt["o_ps"][:, off : off + w],
                        lhsT=qkv_tiles[h][:, 2 * S + KB * j : 2 * S + KB * (j + 1)],
                        rhs=pt[:, col : col + w],
                        start=stt["ocount"] == 0,
                        stop=stt["ocount"] == stt["nmm"] - 1,
                    )
                    stt["ocount"] += 1
                if not tail:
                    for h, s in fins:
                        stt = strip_states[(h, s)]
                        dens.append(
                            fin_den(stt["acc"], stt["pdens"],
                                    last=(h, s) == last_hs)
                        )
                for (h, s), den_ps in zip(fins, dens):
                    stt = strip_states.pop((h, s))
                    fin_norm(h, s, stt["o_ps"], den_ps,
                             last=(h, s) == last_hs)

            for gi, (cap_idx, used, blocks) in enumerate(groups):
                cap = CAPS[cap_idx]
                for col, h, s, j, off, w, diag, pf, sd in blocks:
                    if pf:
                        load_head(h + 1)
                sg = psg.tile([128, cap], f32, tag=f"sg{cap_idx}")
                for col, h, s, j, off, w, diag, pf, sd in blocks:
                    get_state(h, s)
                    qkv_sb = qkv_tiles[h]
                    nc.tensor.matmul(
                        sg[:, col : col + w],
                        lhsT=qkv_sb[:, S + KB * j : S + KB * (j + 1)],
                        rhs=qkv_sb[:, QS * s + off : QS * s + off + w],
                        start=True,
                        stop=True,
                    )
                pt = ptpool.tile([128, cap], bf16, tag=f"pt{cap_idx}")
                # one exp per contiguous written span (bank-alignment bumps
                # can leave uninitialized gaps inside a group)
                spans = []
                for b in blocks:
                    col, w_ = b[0], b[5]
                    if spans and spans[-1][1] == col:
                        spans[-1][1] = col + w_
                    else:
                        spans.append([col, col + w_])
                for c0, c1 in spans:
                    nc.scalar.activation(
                        pt[:, c0:c1], sg[:, c0:c1], Exp, scale=SCALE
                    )
                for col, h, s, j, off, w, diag, pf, sd in blocks:
                    if w < 512:
                        strip_states[(h, s)]["pdens"].append((pt, col, off, w))
                pend_q.append((blocks, pt))
                lag = 2 if gi < len(groups) - 1 else 1
                while len(pend_q) > lag:
                    consume(pend_q.pop(0), tail=gi >= len(groups) - 1)
            while pend_q:
                consume(pend_q.pop(0), tail=True)
    nc.compile()
    return nc


def get_nc():
    if "nc" not in _nc_cache:
        _nc_cache["nc"] = _build_nc()
    return _nc_cache["nc"]


def _build_const():
    import ml_dtypes

    dk = np.arange(128)[:, None]
    c = np.arange(128)[None, :]
    cst = np.empty((128, 256), ml_dtypes.bfloat16)
    cst[:, 0:128] = (dk <= c).astype(ml_dtypes.bfloat16)
    cst[:, 128:256] = 1.0
    return cst


def make_in_maps(qkv):
    import ml_dtypes

    qkv = np.asarray(qkv, dtype=np.float32)
    cst = _build_const()
    in_maps = []
    for core in range(NCORES):
        qkvT = np.empty((HPC, 128, 3 * S), ml_dtypes.bfloat16)
        for i in range(HPC):
            bh = core * HPC + i
            b, h = bh // H, bh % H
            qkvT[i, :, 0:S] = qkv[b, :, 0, h, :].T
            qkvT[i, :, S : 2 * S] = qkv[b, :, 1, h, :].T
            qkvT[i, :, 2 * S : 3 * S] = (
                qkv[b, :, 2, h, :]
                .reshape(S // KB, KB, D)
                .transpose(1, 0, 2)
                .reshape(KB, S)
            )
        in_maps.append({"qkvT": qkvT, "cst": cst})
    return in_maps


def assemble_out(results):
    out = np.empty((B, S, H, D), np.float32)
    for core in range(NCORES):
        oTc = np.asarray(results[core]["oT"], dtype=np.float32)  # [HPC, 128, S]
        for i in range(HPC):
            bh = core * HPC + i
            b, h = bh // H, bh % H
            out[b, :, h, :] = oTc[i].T
    return out


def kernel(qkv):
    from concourse.bass_utils import run_bass_kernel_spmd

    in_maps = make_in_maps(qkv)
    nc = get_nc()
    res = run_bass_kernel_spmd(nc, in_maps, list(range(NCORES)))
    return assemble_out(res.results)
